# revision 26
# baseline (speedup 1.0000x reference)
"""Trainium2 Bass kernel for nn_MlpwithSOMModule (pairwise-concat MLP + max/mask/sum).

Reference computation (B=8, C=4, T=128, D=64, H=128, G=B*C=32):
  entity  = input[:,:,1] -> [G,T,D];  context = input[:,:,0] -> [G,T,D]
  mask    = (context[:,:,0] != 0)                         [G,T]
  x[g,i,j] = concat(context[g,i], entity[g,j])            [G,T,T,2D]
  for l in 0..5: x = tanh(x @ Ws[l] + bs[l])
  score  = (x @ W_out + b_out)[...,0]                     [G,T,T]
  out[g] = sum_i( max_j(score[g,i,j]) * mask[g,i] )       [G]

Sharding: data-parallel over G across 8 cores (4 groups/core); weights
replicated.  On-chip layout is feature-major ([128 features, pairs]) in
half-units of 8 j x 128 i = 1024 pair-columns.

The kernel is activation-bound: ACT (native tanh, ~1.15us/half from PSUM)
and DVE (one custom clipped-poly instruction, ~1.22us/half) must together
chew 6 layers x 64 halves.  Key structural moves vs a naive pipeline:

* All weights are PRE-SCALED on the host by the per-layer clip scale c2
  (u = clip(c2 x)), so the DVE polynomial op needs no extra multiply and
  the ACT engine compensates for free with activation(..., scale=1/c2).
* Layer 0 has rank structure z0 = A[:,i] + B[:,j]; it is evaluated as a
  SINGLE fused DVE custom op p(clip(Src0+Src1)) with stride-0 broadcast
  APs -- no materialized z, no Pool z-build, no separate activation.
* Layers 1-5 split between ACT (tanh w/ scale) and DVE (deg-5 odd poly),
  routed per-event with an even-spreading accumulator: consecutive
  same-engine activations exceed the 4-slot PSUM lookahead, serialize PE
  behind one engine, starve the other, and drop the idling PE to a
  slower p-state.  The per-layer shares favor DVE on L4/L5 (shallow --
  poly error passes through fewer layers) and ACT on L1/L2.
* Group finalization is deferred to the end of the kernel: the
  finalize chain (mask/max/mult/ones-matmul) would otherwise steal a
  PSUM slot mid-rotation and stall PE at every group boundary.
* Everything flows in fp16 (same PE/DVE/ACT speed as bf16, 8x less
  quantization noise).

The per-layer degree-5 odd polynomial: y = u*((u^2+C0)*u^2 + C1),
u = clip(x, -1, 1) on the pre-scaled x -- fitted per layer with an
E[p(x)-tanh(x)] = 0 constraint under the empirical distribution.
"""

import numpy as np
import ml_dtypes

import concourse.bacc as bacc
import concourse.mybir as mybir
import concourse.tile as tile
from concourse.bass_utils import run_bass_kernel_spmd

B, C, T, D = 8, 4, 128, 64
H = 2 * D            # 128
G = B * C            # 32 groups
N_CORES = 8
G_LOC = G // N_CORES   # 4 groups per core
NJ_HALF = 8            # j's per half-unit
NJ_CHUNK = 16          # j's per score chunk (2 halves)
HCOLS = NJ_HALF * T    # 1024 pair-columns per half-unit
N_HALF = T // NJ_HALF  # 16 halves per group
TOT = G_LOC * N_HALF   # 64 half-units per core

F32 = mybir.dt.float32
FP16 = mybir.dt.float16
AF = mybir.ActivationFunctionType
ALU = mybir.AluOpType
AX = mybir.AxisListType

# Per-layer deg-5 odd polynomial tanh fits: y = u*((u^2 + C0)*u^2 + C1),
# u = clip(x, -1, 1) with the scale C2 pre-folded into the layer weights.
POLY = [
    (-2.0335264107580913, 2.0361078340065024, 0.4722274944186211),  # L0
    (-1.9152371825597945, 1.9404534808113383, 0.5056103885173797),  # L1
    (-1.8413539649946290, 1.8820495873666590, 0.5259411223232746),  # L2
    (-1.7039122582092827, 1.8122203752329333, 0.5469380855560303),  # L3
    (-1.6196619589284693, 1.7524016774334317, 0.5688544273376466),  # L4
    (-1.5761136714968840, 1.7342704218648564, 0.5743375062942505),  # L5
]

L0_LEAD = 3
STAGE_STRIDE = 4

# Greedy router cost model (ns) and per-layer DVE reluctance (ns added to
# DVE's projected finish when deciding: shallow layers L4/L5 carry poly
# error through fewer subsequent layers, so they go to DVE more readily).
ACT_NS, DVE_NS, STT_NS, RED_NS = 1127, 1217, 2283, 170
POOL_PAIRS = (4, 10, 16, 22, 28)   # L0 pairs evaluated on the Pool engine
DVE_BIAS = {1: 0, 2: 0, 3: 0, 4: 0, 5: 0}


def _events():
    """Software-pipeline event stream: half hu runs stage t at position
    hu*2 + t*4 (fused L0 leads).  Stages: 1=L0, 2..6=mm+act L1..L5,
    7=score."""
    ev = []
    for hu in range(TOT):
        if hu % 2 == 0:
            ev.append((hu * 2 - L0_LEAD, 1, hu))
        for t in range(2, 8):
            ev.append((hu * 2 + t * STAGE_STRIDE, t, hu))
    ev.sort()
    return ev


def _routing():
    """(l, hu) -> True if DVE-routed.  Greedy balance: track both engines'
    projected finish times (DVE also owns the fused L0s and the score
    reduces) and send each activation to whichever finishes first, with a
    per-layer accuracy bias.  This keeps the queues level through every
    phase of the pipeline (L1/L2-heavy start, L4/L5-heavy drain)."""
    table = {}
    t_act = 0.0
    t_dve = 0.0
    for _pos, t, hu in _events():
        if t == 1:
            if (hu // 2) not in POOL_PAIRS:
                t_dve += STT_NS
        elif t == 7:
            if hu % 2 == 1:
                t_dve += RED_NS
        else:
            l = t - 1
            if hu < 4 or (l == 5 and hu >= 61):
                table[(l, hu)] = False     # warm start / ACT drain
                t_act += ACT_NS
            elif t_dve + DVE_NS + DVE_BIAS[l] <= t_act + ACT_NS:
                table[(l, hu)] = True
                t_dve += DVE_NS
            else:
                table[(l, hu)] = False
                t_act += ACT_NS
    return table


_cached_nc = {}
_ops = None


def _register_ops():
    """Register the two custom DVE ops (idempotent)."""
    global _ops
    if _ops is not None:
        return _ops
    import concourse.dve_ops as DO
    from concourse.dve_spec import Spec, Src0, Src1, C0, C1, Zero, One, \
        sq, maxx, minn, lower
    from concourse.dve_uop import DveOpSpec
    from concourse.dve_table_gen import dve_ver_for
    from concourse.dve_ops import has_src1

    ver = dve_ver_for("TRN2")

    def _clip_poly(x):
        u = maxx(minn(x, One), Zero - One)
        t = sq(u)
        return u * ((t + C0) * t + C1)

    ops = {}
    for name, body in [
        ("TANH_P5_NC2", _clip_poly(Src0)),
        ("TANH_P5_STT", _clip_poly(Src0 + Src1)),
    ]:
        if name in DO._SUB_OPCODE_FOR_NAME:
            ops[name] = [o for o in DO.OPS if o.name == name][0]
            continue
        spec = Spec(body=body)
        row = DO._CUSTOM_DVE_ROW_BASE + len(DO.OPS)
        tmp = DveOpSpec(name=name, opcode=row, uops=lower(spec, ver=ver),
                        rd1_en=has_src1(spec))
        op = DO.DveOp(name, spec, subdim=False, uops_sha={ver: tmp.sha(ver)})
        DO.OPS.append(op)
        DO._SUB_OPCODE_FOR_NAME[name] = row
        DO.CUSTOM_DVE_SPECS[name] = spec
        ops[name] = op
    _ops = ops
    return ops


def _build_program(bias_zero):
    ops = _register_ops()
    op_ttss = ops["TANH_P5_NC2"]
    op_stt = ops["TANH_P5_STT"]
    nc = bacc.Bacc("TRN2", target_bir_lowering=False, debug=False,
                   num_devices=N_CORES)

    gin_d = nc.dram_tensor("gin", [D, 2 * G_LOC * T], FP16, kind="ExternalInput")
    ctx0_d = nc.dram_tensor("ctx0", [T, G_LOC], F32, kind="ExternalInput")
    ws_d = nc.dram_tensor("Ws", [6, H, H], FP16, kind="ExternalInput")
    w0b_d = nc.dram_tensor("w0b", [D, H], FP16, kind="ExternalInput")
    bsT_d = nc.dram_tensor("bsT", [H, 6], F32, kind="ExternalInput")
    bs0s_d = nc.dram_tensor("bs0s", [H, 1], F32, kind="ExternalInput")
    bsrow_d = nc.dram_tensor("bsrow", [1, 6 * H], FP16, kind="ExternalInput")
    wout_d = nc.dram_tensor("wout", [H, 1], FP16, kind="ExternalInput")
    bout_d = nc.dram_tensor("bout", [T, 1], F32, kind="ExternalInput")
    out_d = nc.dram_tensor("out", [G_LOC, 1], F32, kind="ExternalOutput")

    from concourse.bass import broadcast_tensor_aps

    with tile.TileContext(nc) as tc:
        with (
            tc.tile_pool(name="consts", bufs=1) as consts,
            tc.tile_pool(name="hpool", bufs=20) as hpool,
            tc.tile_pool(name="h0pool", bufs=8) as h0pool,
            tc.tile_pool(name="chpool", bufs=6) as chpool,
            tc.tile_pool(name="ctmp", bufs=2) as ctmp,
            tc.tile_pool(name="small", bufs=8) as small,
            tc.tile_pool(name="psum", bufs=4, space="PSUM") as psum,
        ):
            # dummy activation first: pulls the tanh ACT_TABLE_LOAD (~1.3us)
            # off the critical path, overlapping it with setup DMAs
            scratch_sb = consts.tile([1, 1], F32)
            scratch2_sb = consts.tile([1, 1], F32)
            nc.gpsimd.memset(scratch_sb[:], 0.0)
            nc.scalar.activation(scratch2_sb[:], scratch_sb[:], AF.Tanh)


            ws_sb = consts.tile([H, 6 * H], FP16)
            bsT_sb = consts.tile([H, 6], F32)
            bs0s_sb = consts.tile([H, 1], F32)
            gin_sb = consts.tile([D, 2 * G_LOC * T], FP16)
            ctx0all_sb = consts.tile([T, G_LOC], F32)
            # layer-0 prerequisites first so half 0 can start ASAP; group 0's
            # slice of gin is a separate DMA so its dependency fires early
            w0b_sb = consts.tile([D, H], FP16)
            for _g in range(G_LOC):
                nc.sync.dma_start(gin_sb[:, _g * 2 * T:(_g + 1) * 2 * T],
                                  gin_d[:, _g * 2 * T:(_g + 1) * 2 * T])
            nc.gpsimd.dma_start(ws_sb[:, 0:H], ws_d[0])
            nc.gpsimd.dma_start(w0b_sb[:], w0b_d[:])
            nc.gpsimd.dma_start(ctx0all_sb[:], ctx0_d[:])
            nc.gpsimd.dma_start(bsT_sb[:], bsT_d[:])
            nc.gpsimd.dma_start(bs0s_sb[:], bs0s_d[:])
            wout_sb = consts.tile([H, 1], FP16)
            bout_sb = consts.tile([T, 1], F32)
            ones_sb = consts.tile([T, 1], F32)
            res_sb = consts.tile([G_LOC, 1], F32)
            mmall_sb = consts.tile([T, G_LOC], F32)
            bsrow_sb = consts.tile([1, 6 * H], FP16)
            nc.gpsimd.dma_start(bsrow_sb[:], bsrow_d[:])
            ones512_sb = consts.tile([1, 512], FP16)
            nc.vector.memset(ones512_sb[:], 1.0)

            # Per-group setup: feature-major A/Bb (fp16, c2_0 pre-scaled via
            # the host-scaled W0) for the fused layer-0 op.
            a_sbs = [None] * G_LOC
            bb_sbs = [None] * G_LOC
            rmax_sbs = [None] * G_LOC
            mask_sbs = [None] * G_LOC

            def setup_group(g):
                ctxT_sl = gin_sb[:, (2 * g) * T:(2 * g + 1) * T]
                entT_sl = gin_sb[:, (2 * g + 1) * T:(2 * g + 2) * T]
                # A = (ctx @ W0_top).T : [H, T(i)];  Bb = (ent @ W0_bot).T + c2*b0
                ps_a = psum.tile([H, HCOLS], F32, tag="mm")
                nc.tensor.matmul(ps_a[:, 0:T], ws_sb[0:D, 0:H], ctxT_sl,
                                 start=True, stop=True)
                a_sb = consts.tile([H, T], FP16, tag=f"a{g}")
                nc.vector.tensor_copy(a_sb[:], ps_a[:, 0:T])
                ps_b = psum.tile([H, HCOLS], F32, tag="mm")
                nc.tensor.matmul(ps_b[:, 0:T], w0b_sb[:], entT_sl,
                                 start=True, stop=True)
                bb_sb = consts.tile([H, T], FP16, tag=f"bb{g}")
                if bias_zero:
                    nc.vector.tensor_copy(bb_sb[:], ps_b[:, 0:T])
                else:
                    nc.vector.tensor_scalar_add(bb_sb[:], ps_b[:, 0:T],
                                                bs0s_sb[:, 0:1])
                rmax_sb = consts.tile([T, NJ_HALF], F32, tag=f"rmax{g}")
                a_sbs[g], bb_sbs[g] = a_sb, bb_sb
                rmax_sbs[g] = rmax_sb

            ps_cur = {}
            h_cur = {}
            route = _routing()
            c0v_sb = consts.tile([H, 1], FP16)
            c1v_sb = consts.tile([H, 1], FP16)
            nc.vector.memset(c0v_sb[:], float(POLY[0][0]))
            nc.vector.memset(c1v_sb[:], float(POLY[0][1]))

            def stage_l0_pool(hu):
                """Fused layer 0 for one pair of halves on the (otherwise
                idle) Pool engine: 7 tensor_tensor/tensor_scalar ops compute
                p(clip(A+B)) entirely in SBUF, relieving the DVE."""
                g, ch = hu // N_HALF, hu % N_HALF
                h_sb = chpool.tile([H, 2 * HCOLS], FP16, tag="hp")
                a_ap = a_sbs[g][:].rearrange("p (o i) -> p o i", o=1)
                b_ap = bb_sbs[g][:, ch * NJ_HALF:(ch + 2) * NJ_HALF] \
                    .rearrange("p (j o) -> p j o", o=1)
                a_b, b_b = broadcast_tensor_aps(a_ap, b_ap)
                z_sb = ctmp.tile([H, 2 * HCOLS], FP16, tag="z")
                u_sb = ctmp.tile([H, 2 * HCOLS], FP16, tag="u")
                t_sb = ctmp.tile([H, 2 * HCOLS], FP16, tag="t")
                v_sb = ctmp.tile([H, 2 * HCOLS], FP16, tag="v")
                z_ap = z_sb[:].rearrange("p (j i) -> p j i", j=2 * NJ_HALF)
                nc.gpsimd.tensor_tensor(z_ap, a_b, b_b, op=ALU.add)
                nc.gpsimd.tensor_scalar(u_sb[:], z_sb[:], 1.0, -1.0,
                                        op0=ALU.min, op1=ALU.max)
                nc.gpsimd.tensor_tensor(t_sb[:], u_sb[:], u_sb[:], op=ALU.mult)
                t_ap = t_sb[:].rearrange("p (o c) -> p o c", o=1)
                c0_ap = c0v_sb[:].rearrange("p (o c) -> p o c", o=1)
                t_b, c0_b = broadcast_tensor_aps(t_ap, c0_ap)
                v_ap = v_sb[:].rearrange("p (o c) -> p o c", o=1)
                nc.gpsimd.tensor_tensor(v_ap, t_b, c0_b, op=ALU.add)
                nc.gpsimd.tensor_tensor(v_sb[:], v_sb[:], t_sb[:], op=ALU.mult)
                c1_ap = c1v_sb[:].rearrange("p (o c) -> p o c", o=1)
                v_b, c1_b = broadcast_tensor_aps(v_ap, c1_ap)
                nc.gpsimd.tensor_tensor(v_ap, v_b, c1_b, op=ALU.add)
                nc.gpsimd.tensor_tensor(h_sb[:], v_sb[:], u_sb[:], op=ALU.mult)
                h_cur[hu] = h_sb[:, 0:HCOLS]
                h_cur[hu + 1] = h_sb[:, HCOLS:2 * HCOLS]

            def stage_l0(hu):
                """Fused layer 0 on DVE for a PAIR of halves (hu even):
                h = p(clip(A[:,i] + Bb[:,j])) in one 2048-col custom STT
                instruction with broadcast APs (SBUF-only, so the wide
                instruction costs no extra PSUM)."""
                g, ch = hu // N_HALF, hu % N_HALF
                h_sb = h0pool.tile([H, 2 * HCOLS], FP16, tag="h0")
                a_ap = a_sbs[g][:].rearrange("p (o i) -> p o i", o=1)
                b_ap = bb_sbs[g][:, ch * NJ_HALF:(ch + 2) * NJ_HALF] \
                    .rearrange("p (j o) -> p j o", o=1)
                a_b, b_b = broadcast_tensor_aps(a_ap, b_ap)
                h_ap = h_sb[:].rearrange("p (j i) -> p j i", j=2 * NJ_HALF)
                c0, c1, _ = POLY[0]
                nc.vector._custom_dve(op_stt, out=h_ap, in0=a_b, in1=b_b,
                                      s0=c0, s1=c1)
                h_cur[hu] = h_sb[:, 0:HCOLS]
                h_cur[hu + 1] = h_sb[:, HCOLS:2 * HCOLS]

            def stage_act(hu, l):
                """Layer-l (l>=1) activation from PSUM: ACT native tanh with
                scale=1/c2, or the DVE custom poly (weights pre-scaled)."""
                ps = ps_cur.pop(hu)
                h_tile = hpool.tile([H, HCOLS], FP16, tag="h")
                h_sb = h_tile[:]
                c0, c1, c2 = POLY[l]
                if route[(l, hu)]:
                    nc.vector._custom_dve(op_ttss, out=h_sb, in0=ps[:],
                                          s0=c0, s1=c1)
                elif bias_zero:
                    nc.scalar.activation(h_sb, ps[:], AF.Tanh,
                                         scale=1.0 / c2)
                else:
                    nc.scalar.activation(h_sb, ps[:], AF.Tanh,
                                         scale=1.0 / c2,
                                         bias=bsT_sb[:, l:l + 1])
                h_cur[hu] = h_sb

            def stage_mm(hu, l):
                """Layer-l matmul (l>=1) for half hu; for DVE-routed halves
                with nonzero bias, accumulate the c2-scaled bias row."""
                ht = h_cur[hu]
                ps = psum.tile([H, HCOLS], F32, tag="mm")
                need_bias_mm = (not bias_zero) and route[(l, hu)]
                for q in range(2):
                    sl = slice(q * 512, (q + 1) * 512)
                    if need_bias_mm:
                        nc.tensor.matmul(ps[:, sl],
                                         bsrow_sb[0:1, l * H:(l + 1) * H],
                                         ones512_sb[:], start=True, stop=False)
                    nc.tensor.matmul(ps[:, sl], ws_sb[:, l * H:(l + 1) * H],
                                     ht[:, sl], start=not need_bias_mm,
                                     stop=True)
                ps_cur[hu] = ps

            sc_cur = {}

            def stage_score(hu):
                """Final layer: per j-block stationary-h matmuls into a
                per-chunk [T, 16] tile; on the odd half also do the 16-col
                DVE max-reduce into this group's rmax column."""
                g = hu // N_HALF
                ht = h_cur.pop(hu)
                odd = hu % 2
                if not odd:
                    sc_new = psum.tile([H, HCOLS], F32, tag="mm")
                    sc_cur[hu] = sc_new
                sc = sc_cur[hu - odd]
                for jl in range(NJ_HALF):
                    nc.tensor.matmul(sc[:, odd * NJ_HALF + jl:odd * NJ_HALF + jl + 1],
                                     ht[:, jl * T:(jl + 1) * T],
                                     wout_sb[:], start=True, stop=True)
                if odd:
                    sc = sc_cur.pop(hu - 1)
                    ch = (hu // 2) % NJ_HALF
                    nc.vector.tensor_reduce(rmax_sbs[g][:, ch:ch + 1],
                                            sc[:, 0:NJ_CHUNK],
                                            axis=AX.X, op=ALU.max)

            def make_mask(g):
                # mask = (ctx[:,0] != 0), computed upfront (only needs ctx0)
                mask_sb = small.tile([T, 1], F32, tag=f"mask{g}")
                nc.vector.tensor_scalar(mask_sb[:], ctx0all_sb[:, g:g + 1],
                                        0.0, None, op0=ALU.not_equal)
                mask_sbs[g] = mask_sb

            def finalize_group(g):
                # mmall[:,g] = (max_ch rmax + b_out) * mask, deferred to the
                # end so the chain never steals a PSUM slot mid-rotation
                rmx_sb = small.tile([T, 1], F32, tag="rmx")
                nc.vector.tensor_reduce(rmx_sb[:], rmax_sbs[g][:],
                                        axis=AX.X, op=ALU.max)
                nc.vector.scalar_tensor_tensor(mmall_sb[:, g:g + 1], rmx_sb[:],
                                               bout_sb[:, 0:1], mask_sbs[g][:],
                                               op0=ALU.add, op1=ALU.mult)

            def finalize_sum():
                # partition-axis sums via one ones-matmul: [G_LOC,1]
                sum_ps = psum.tile([H, HCOLS], F32, tag="mm")
                nc.tensor.matmul(sum_ps[0:G_LOC, 0:1], mmall_sb[:], ones_sb[:],
                                 start=True, stop=True)
                nc.vector.tensor_copy(res_sb[:], sum_ps[0:G_LOC, 0:1])

            # group 0 + first fused-L0s go first so the pipeline starts ASAP;
            # the rest of the setup DMAs overlap with the first halves
            setup_group(0)
            stage_l0(0)
            setup_group(1)
            stage_l0(2)
            setup_group(2)
            setup_group(3)
            for _pu in POOL_PAIRS:
                stage_l0_pool(2 * _pu)
            nc.sync.dma_start(
                ws_sb[:, H:6 * H].rearrange("p (l c) -> p l c", l=5),
                ws_d[1:6].rearrange("l p c -> p l c"))
            nc.gpsimd.dma_start(wout_sb[:], wout_d[:])
            nc.gpsimd.dma_start(bout_sb[:], bout_d[:])
            nc.vector.memset(ones_sb[:], 1.0)
            for g in range(G_LOC):
                make_mask(g)

            for _pos, t, hu in _events():
                if t == 1:
                    if hu >= 4 and (hu // 2) not in POOL_PAIRS:
                        stage_l0(hu)
                elif t <= 6:
                    stage_mm(hu, t - 1)
                    stage_act(hu, t - 1)
                else:
                    stage_score(hu)

            for g in range(G_LOC):
                finalize_group(g)
            finalize_sum()
            nc.sync.dma_start(out_d[:], res_sb[:])

    nc.compile()
    return nc


def _get_nc(bias_zero):
    if bias_zero not in _cached_nc:
        _cached_nc[bias_zero] = _build_program(bias_zero)
    return _cached_nc[bias_zero]


def _fp16(a):
    return np.ascontiguousarray(a.astype(np.float16))


def _prep_in_maps(input, Ws, bs, W_out, b_out):
    input = np.ascontiguousarray(np.asarray(input, dtype=np.float32))
    Ws = np.asarray(Ws, dtype=np.float32)
    bs = np.asarray(bs, dtype=np.float32)
    W_out = np.asarray(W_out, dtype=np.float32)
    b_out = np.asarray(b_out, dtype=np.float32)

    c2 = np.array([p[2] for p in POLY], dtype=np.float32)  # per-layer clip scale
    WsS = Ws * c2[:, None, None]       # pre-scaled weights (fp16 on device)

    ctx = input[:, :, 0].reshape(G, T, D)
    ent = input[:, :, 1].reshape(G, T, D)
    ctxT = _fp16(ctx.transpose(0, 2, 1))                  # [G, D, T]
    entT = _fp16(ent.transpose(0, 2, 1))
    ctx0 = np.ascontiguousarray(ctx[:, :, 0])             # [G, T] fp32
    ws16 = _fp16(WsS)
    w0b = _fp16(WsS[0][D:H])
    bsT = np.ascontiguousarray(bs.T)                      # [H, 6] raw (ACT bias)
    bs0s = np.ascontiguousarray((bs[0] * c2[0]).reshape(H, 1))
    bsrow = _fp16((bs * c2[:, None]).reshape(1, 6 * H))   # scaled (DVE mm bias)
    wout = _fp16(W_out)
    bout = np.broadcast_to(b_out.reshape(1, 1), (T, 1)).copy()

    in_maps = []
    for k in range(N_CORES):
        sl = slice(k * G_LOC, (k + 1) * G_LOC)
        gin = np.empty((D, 2 * G_LOC * T), dtype=ctxT.dtype)
        for g in range(G_LOC):
            gin[:, (2 * g) * T:(2 * g + 1) * T] = ctxT[k * G_LOC + g]
            gin[:, (2 * g + 1) * T:(2 * g + 2) * T] = entT[k * G_LOC + g]
        in_maps.append({
            "gin": np.ascontiguousarray(gin),
            "ctx0": np.ascontiguousarray(ctx0[sl].T),
            "Ws": ws16,
            "w0b": w0b,
            "bsT": bsT,
            "bs0s": bs0s,
            "bsrow": bsrow,
            "wout": wout,
            "bout": bout,
        })
    return in_maps


def run_traced(trace=False, **inputs):
    """Returns (output [G], exec_time_ns or None)."""
    nc = _get_nc(bias_zero=bool(np.all(np.asarray(inputs["bs"]) == 0)
                                and np.all(np.asarray(inputs["b_out"]) == 0)))
    in_maps = _prep_in_maps(**inputs)
    res = run_bass_kernel_spmd(nc, in_maps, list(range(N_CORES)), trace=trace)
    out = np.concatenate([res.results[k]["out"].reshape(G_LOC)
                          for k in range(N_CORES)])
    return out, res.exec_time_ns


def kernel(**inputs) -> np.ndarray:
    # One rare device-level flake was observed to corrupt a single run, so
    # verify by agreement: run twice; on mismatch run a third time and take
    # the elementwise median.
    out1, _ = run_traced(trace=False, **inputs)
    out2, _ = run_traced(trace=False, **inputs)
    if np.allclose(out1, out2, rtol=1e-4, atol=1e-3):
        return out1
    out3, _ = run_traced(trace=False, **inputs)
    return np.median(np.stack([out1, out2, out3]), axis=0).astype(out1.dtype)


# revision 27
# speedup vs baseline: 1.2617x; 1.2617x over previous
"""Trainium2 Bass kernel for nn_MlpwithSOMModule (pairwise-concat MLP + max/mask/sum).

Reference computation (B=8, C=4, T=128, D=64, H=128, G=B*C=32):
  entity  = input[:,:,1] -> [G,T,D];  context = input[:,:,0] -> [G,T,D]
  mask    = (context[:,:,0] != 0)                         [G,T]
  x[g,i,j] = concat(context[g,i], entity[g,j])            [G,T,T,2D]
  for l in 0..5: x = tanh(x @ Ws[l] + bs[l])
  score  = (x @ W_out + b_out)[...,0]                     [G,T,T]
  out[g] = sum_i( max_j(score[g,i,j]) * mask[g,i] )       [G]

Sharding: data-parallel over G across 8 cores (4 groups/core); weights
replicated.  On-chip layout is feature-major ([128 features, pairs]) in
half-units of 8 j x 128 i = 1024 pair-columns.

The kernel is activation-bound: ACT (native tanh, ~1.15us/half from PSUM)
and DVE (one custom clipped-poly instruction, ~1.22us/half) must together
chew 6 layers x 64 halves.  Key structural moves vs a naive pipeline:

* All weights are PRE-SCALED on the host by the per-layer clip scale c2
  (u = clip(c2 x)), so the DVE polynomial op needs no extra multiply and
  the ACT engine compensates for free with activation(..., scale=1/c2).
* Layer 0 has rank structure z0 = A[:,i] + B[:,j]; it is evaluated as a
  SINGLE fused DVE custom op p(clip(Src0+Src1)) with stride-0 broadcast
  APs -- no materialized z, no Pool z-build, no separate activation.
* Layers 1-5 split between ACT (tanh w/ scale) and DVE (deg-5 odd poly),
  routed per-event with an even-spreading accumulator: consecutive
  same-engine activations exceed the 4-slot PSUM lookahead, serialize PE
  behind one engine, starve the other, and drop the idling PE to a
  slower p-state.  The per-layer shares favor DVE on L4/L5 (shallow --
  poly error passes through fewer layers) and ACT on L1/L2.
* Group finalization is deferred to the end of the kernel: the
  finalize chain (mask/max/mult/ones-matmul) would otherwise steal a
  PSUM slot mid-rotation and stall PE at every group boundary.
* Everything flows in fp16 (same PE/DVE/ACT speed as bf16, 8x less
  quantization noise).

The per-layer degree-5 odd polynomial: y = u*((u^2+C0)*u^2 + C1),
u = clip(x, -1, 1) on the pre-scaled x -- fitted per layer with an
E[p(x)-tanh(x)] = 0 constraint under the empirical distribution.
"""

import numpy as np
import ml_dtypes

import concourse.bacc as bacc
import concourse.mybir as mybir
import concourse.tile as tile
from concourse.bass_utils import run_bass_kernel_spmd

B, C, T, D = 8, 4, 128, 64
H = 2 * D            # 128
G = B * C            # 32 groups
N_CORES = 8
G_LOC = G // N_CORES   # 4 groups per core
NJ_HALF = 8            # j's per half-unit
NJ_CHUNK = 16          # j's per score chunk (2 halves)
HCOLS = NJ_HALF * T    # 1024 pair-columns per half-unit
N_HALF = T // NJ_HALF  # 16 halves per group
TOT = G_LOC * N_HALF   # 64 half-units per core

F32 = mybir.dt.float32
FP16 = mybir.dt.float16
AF = mybir.ActivationFunctionType
ALU = mybir.AluOpType
AX = mybir.AxisListType

# Per-layer deg-5 odd polynomial tanh fits: y = u*((u^2 + C0)*u^2 + C1),
# u = clip(x, -1, 1) with the scale C2 pre-folded into the layer weights.
POLY = [
    (-2.0335264107580913, 2.0361078340065024, 0.4722274944186211),  # L0
    (-1.9152371825597945, 1.9404534808113383, 0.5056103885173797),  # L1
    (-1.8413539649946290, 1.8820495873666590, 0.5259411223232746),  # L2
    (-1.7039122582092827, 1.8122203752329333, 0.5469380855560303),  # L3
    (-1.6196619589284693, 1.7524016774334317, 0.5688544273376466),  # L4
    (-1.5761136714968840, 1.7342704218648564, 0.5743375062942505),  # L5
]

L0_LEAD = 3
STAGE_STRIDE = 4

# Greedy router cost model (ns) and per-layer DVE reluctance (ns added to
# DVE's projected finish when deciding: shallow layers L4/L5 carry poly
# error through fewer subsequent layers, so they go to DVE more readily).
ACT_NS, DVE_NS, STT_NS, RED_NS = 1127, 1217, 2283, 170
DVE_BIAS = {1: 0, 2: 0, 3: 0, 4: 0, 5: 0}


def _events():
    """Software-pipeline event stream: half hu runs stage t at position
    hu*2 + t*4 (fused L0 leads).  Stages: 1=L0, 2..6=mm+act L1..L5,
    7=score."""
    ev = []
    for hu in range(TOT):
        if hu % 2 == 0:
            ev.append((hu * 2 - L0_LEAD, 1, hu))
        for t in range(2, 8):
            ev.append((hu * 2 + t * STAGE_STRIDE, t, hu))
    ev.sort()
    return ev


def _routing():
    """(l, hu) -> True if DVE-routed.  Greedy balance: track both engines'
    projected finish times (DVE also owns the fused L0s and the score
    reduces) and send each activation to whichever finishes first, with a
    per-layer accuracy bias.  This keeps the queues level through every
    phase of the pipeline (L1/L2-heavy start, L4/L5-heavy drain)."""
    table = {}
    t_act = 0.0
    t_dve = 0.0
    for _pos, t, hu in _events():
        if t == 1:
            t_dve += STT_NS
        elif t == 7:
            if hu % 2 == 1:
                t_dve += RED_NS
        else:
            l = t - 1
            if hu < 4 or (l == 5 and hu >= 61):
                table[(l, hu)] = False     # warm start / ACT drain
                t_act += ACT_NS
            elif t_dve + DVE_NS + DVE_BIAS[l] <= t_act + ACT_NS:
                table[(l, hu)] = True
                t_dve += DVE_NS
            else:
                table[(l, hu)] = False
                t_act += ACT_NS
    return table


_cached_nc = {}
_ops = None


def _register_ops():
    """Register the two custom DVE ops (idempotent)."""
    global _ops
    if _ops is not None:
        return _ops
    import concourse.dve_ops as DO
    from concourse.dve_spec import Spec, Src0, Src1, C0, C1, Zero, One, \
        sq, maxx, minn, lower
    from concourse.dve_uop import DveOpSpec
    from concourse.dve_table_gen import dve_ver_for
    from concourse.dve_ops import has_src1

    ver = dve_ver_for("TRN2")

    def _clip_poly(x):
        u = maxx(minn(x, One), Zero - One)
        t = sq(u)
        return u * ((t + C0) * t + C1)

    ops = {}
    for name, body in [
        ("TANH_P5_NC2", _clip_poly(Src0)),
        ("TANH_P5_STT", _clip_poly(Src0 + Src1)),
    ]:
        if name in DO._SUB_OPCODE_FOR_NAME:
            ops[name] = [o for o in DO.OPS if o.name == name][0]
            continue
        spec = Spec(body=body)
        row = DO._CUSTOM_DVE_ROW_BASE + len(DO.OPS)
        tmp = DveOpSpec(name=name, opcode=row, uops=lower(spec, ver=ver),
                        rd1_en=has_src1(spec))
        op = DO.DveOp(name, spec, subdim=False, uops_sha={ver: tmp.sha(ver)})
        DO.OPS.append(op)
        DO._SUB_OPCODE_FOR_NAME[name] = row
        DO.CUSTOM_DVE_SPECS[name] = spec
        ops[name] = op
    _ops = ops
    return ops


def _build_program(bias_zero):
    ops = _register_ops()
    op_ttss = ops["TANH_P5_NC2"]
    op_stt = ops["TANH_P5_STT"]
    nc = bacc.Bacc("TRN2", target_bir_lowering=False, debug=False,
                   num_devices=N_CORES)

    gin_d = nc.dram_tensor("gin", [D, 2 * G_LOC * T], FP16, kind="ExternalInput")
    ctx0_d = nc.dram_tensor("ctx0", [T, G_LOC], F32, kind="ExternalInput")
    ws_d = nc.dram_tensor("Ws", [6, H, H], FP16, kind="ExternalInput")
    w0b_d = nc.dram_tensor("w0b", [D, H], FP16, kind="ExternalInput")
    bsT_d = nc.dram_tensor("bsT", [H, 6], F32, kind="ExternalInput")
    bs0s_d = nc.dram_tensor("bs0s", [H, 1], F32, kind="ExternalInput")
    bsrow_d = nc.dram_tensor("bsrow", [1, 6 * H], FP16, kind="ExternalInput")
    wout_d = nc.dram_tensor("wout", [H, 1], FP16, kind="ExternalInput")
    bout_d = nc.dram_tensor("bout", [T, 1], F32, kind="ExternalInput")
    out_d = nc.dram_tensor("out", [G_LOC, 1], F32, kind="ExternalOutput")

    from concourse.bass import broadcast_tensor_aps

    with tile.TileContext(nc) as tc:
        with (
            tc.tile_pool(name="consts", bufs=1) as consts,
            tc.tile_pool(name="hpool", bufs=20) as hpool,
            tc.tile_pool(name="h0pool", bufs=8) as h0pool,
            tc.tile_pool(name="small", bufs=8) as small,
            tc.tile_pool(name="psum", bufs=4, space="PSUM") as psum,
        ):
            # dummy activation first: pulls the tanh ACT_TABLE_LOAD (~1.3us)
            # off the critical path, overlapping it with setup DMAs
            scratch_sb = consts.tile([1, 1], F32)
            scratch2_sb = consts.tile([1, 1], F32)
            nc.gpsimd.memset(scratch_sb[:], 0.0)
            nc.scalar.activation(scratch2_sb[:], scratch_sb[:], AF.Tanh)


            ws_sb = consts.tile([H, 6 * H], FP16)
            bsT_sb = consts.tile([H, 6], F32)
            bs0s_sb = consts.tile([H, 1], F32)
            gin_sb = consts.tile([D, 2 * G_LOC * T], FP16)
            ctx0all_sb = consts.tile([T, G_LOC], F32)
            # layer-0 prerequisites first so half 0 can start ASAP; group 0's
            # slice of gin is a separate DMA so its dependency fires early
            w0b_sb = consts.tile([D, H], FP16)
            for _g in range(G_LOC):
                nc.sync.dma_start(gin_sb[:, _g * 2 * T:(_g + 1) * 2 * T],
                                  gin_d[:, _g * 2 * T:(_g + 1) * 2 * T])
            nc.gpsimd.dma_start(ws_sb[:, 0:H], ws_d[0])
            nc.gpsimd.dma_start(w0b_sb[:], w0b_d[:])
            nc.gpsimd.dma_start(ctx0all_sb[:], ctx0_d[:])
            nc.gpsimd.dma_start(bsT_sb[:], bsT_d[:])
            nc.gpsimd.dma_start(bs0s_sb[:], bs0s_d[:])
            wout_sb = consts.tile([H, 1], FP16)
            bout_sb = consts.tile([T, 1], F32)
            ones_sb = consts.tile([T, 1], F32)
            res_sb = consts.tile([G_LOC, 1], F32)
            mmall_sb = consts.tile([T, G_LOC], F32)
            bsrow_sb = consts.tile([1, 6 * H], FP16)
            nc.gpsimd.dma_start(bsrow_sb[:], bsrow_d[:])
            ones512_sb = consts.tile([1, 512], FP16)
            nc.vector.memset(ones512_sb[:], 1.0)

            # Per-group setup: feature-major A/Bb (fp16, c2_0 pre-scaled via
            # the host-scaled W0) for the fused layer-0 op.
            a_sbs = [None] * G_LOC
            bb_sbs = [None] * G_LOC
            rmax_sbs = [None] * G_LOC
            mask_sbs = [None] * G_LOC

            def setup_group(g):
                ctxT_sl = gin_sb[:, (2 * g) * T:(2 * g + 1) * T]
                entT_sl = gin_sb[:, (2 * g + 1) * T:(2 * g + 2) * T]
                # A = (ctx @ W0_top).T : [H, T(i)];  Bb = (ent @ W0_bot).T + c2*b0
                ps_a = psum.tile([H, HCOLS], F32, tag="mm")
                nc.tensor.matmul(ps_a[:, 0:T], ws_sb[0:D, 0:H], ctxT_sl,
                                 start=True, stop=True)
                a_sb = consts.tile([H, T], FP16, tag=f"a{g}")
                nc.vector.tensor_copy(a_sb[:], ps_a[:, 0:T])
                ps_b = psum.tile([H, HCOLS], F32, tag="mm")
                nc.tensor.matmul(ps_b[:, 0:T], w0b_sb[:], entT_sl,
                                 start=True, stop=True)
                bb_sb = consts.tile([H, T], FP16, tag=f"bb{g}")
                if bias_zero:
                    nc.vector.tensor_copy(bb_sb[:], ps_b[:, 0:T])
                else:
                    nc.vector.tensor_scalar_add(bb_sb[:], ps_b[:, 0:T],
                                                bs0s_sb[:, 0:1])
                rmax_sb = consts.tile([T, NJ_HALF], F32, tag=f"rmax{g}")
                a_sbs[g], bb_sbs[g] = a_sb, bb_sb
                rmax_sbs[g] = rmax_sb

            ps_cur = {}
            h_cur = {}
            route = _routing()

            def stage_l0(hu):
                """Fused layer 0 on DVE for a PAIR of halves (hu even):
                h = p(clip(A[:,i] + Bb[:,j])) in one 2048-col custom STT
                instruction with broadcast APs (SBUF-only, so the wide
                instruction costs no extra PSUM)."""
                g, ch = hu // N_HALF, hu % N_HALF
                h_sb = h0pool.tile([H, 2 * HCOLS], FP16, tag="h0")
                a_ap = a_sbs[g][:].rearrange("p (o i) -> p o i", o=1)
                b_ap = bb_sbs[g][:, ch * NJ_HALF:(ch + 2) * NJ_HALF] \
                    .rearrange("p (j o) -> p j o", o=1)
                a_b, b_b = broadcast_tensor_aps(a_ap, b_ap)
                h_ap = h_sb[:].rearrange("p (j i) -> p j i", j=2 * NJ_HALF)
                c0, c1, _ = POLY[0]
                nc.vector._custom_dve(op_stt, out=h_ap, in0=a_b, in1=b_b,
                                      s0=c0, s1=c1)
                h_cur[hu] = h_sb[:, 0:HCOLS]
                h_cur[hu + 1] = h_sb[:, HCOLS:2 * HCOLS]

            def stage_act(hu, l):
                """Layer-l (l>=1) activation from PSUM: ACT native tanh with
                scale=1/c2, or the DVE custom poly (weights pre-scaled)."""
                ps = ps_cur.pop(hu)
                h_tile = hpool.tile([H, HCOLS], FP16, tag="h")
                h_sb = h_tile[:]
                c0, c1, c2 = POLY[l]
                if route[(l, hu)]:
                    nc.vector._custom_dve(op_ttss, out=h_sb, in0=ps[:],
                                          s0=c0, s1=c1)
                elif bias_zero:
                    nc.scalar.activation(h_sb, ps[:], AF.Tanh,
                                         scale=1.0 / c2)
                else:
                    nc.scalar.activation(h_sb, ps[:], AF.Tanh,
                                         scale=1.0 / c2,
                                         bias=bsT_sb[:, l:l + 1])
                h_cur[hu] = h_sb

            def stage_mm(hu, l):
                """Layer-l matmul (l>=1) for half hu; for DVE-routed halves
                with nonzero bias, accumulate the c2-scaled bias row."""
                ht = h_cur[hu]
                ps = psum.tile([H, HCOLS], F32, tag="mm")
                need_bias_mm = (not bias_zero) and route[(l, hu)]
                for q in range(2):
                    sl = slice(q * 512, (q + 1) * 512)
                    if need_bias_mm:
                        nc.tensor.matmul(ps[:, sl],
                                         bsrow_sb[0:1, l * H:(l + 1) * H],
                                         ones512_sb[:], start=True, stop=False)
                    nc.tensor.matmul(ps[:, sl], ws_sb[:, l * H:(l + 1) * H],
                                     ht[:, sl], start=not need_bias_mm,
                                     stop=True)
                ps_cur[hu] = ps

            sc_cur = {}

            def stage_score(hu):
                """Final layer: per j-block stationary-h matmuls into a
                per-chunk [T, 16] tile; on the odd half also do the 16-col
                DVE max-reduce into this group's rmax column."""
                g = hu // N_HALF
                ht = h_cur.pop(hu)
                odd = hu % 2
                if not odd:
                    sc_new = psum.tile([H, HCOLS], F32, tag="mm")
                    sc_cur[hu] = sc_new
                sc = sc_cur[hu - odd]
                for jl in range(NJ_HALF):
                    nc.tensor.matmul(sc[:, odd * NJ_HALF + jl:odd * NJ_HALF + jl + 1],
                                     ht[:, jl * T:(jl + 1) * T],
                                     wout_sb[:], start=True, stop=True)
                if odd:
                    sc = sc_cur.pop(hu - 1)
                    ch = (hu // 2) % NJ_HALF
                    nc.vector.tensor_reduce(rmax_sbs[g][:, ch:ch + 1],
                                            sc[:, 0:NJ_CHUNK],
                                            axis=AX.X, op=ALU.max)

            def make_mask(g):
                # mask = (ctx[:,0] != 0), computed upfront (only needs ctx0)
                mask_sb = small.tile([T, 1], F32, tag=f"mask{g}")
                nc.vector.tensor_scalar(mask_sb[:], ctx0all_sb[:, g:g + 1],
                                        0.0, None, op0=ALU.not_equal)
                mask_sbs[g] = mask_sb

            def finalize_group(g):
                # mmall[:,g] = (max_ch rmax + b_out) * mask, deferred to the
                # end so the chain never steals a PSUM slot mid-rotation
                rmx_sb = small.tile([T, 1], F32, tag="rmx")
                nc.vector.tensor_reduce(rmx_sb[:], rmax_sbs[g][:],
                                        axis=AX.X, op=ALU.max)
                nc.vector.scalar_tensor_tensor(mmall_sb[:, g:g + 1], rmx_sb[:],
                                               bout_sb[:, 0:1], mask_sbs[g][:],
                                               op0=ALU.add, op1=ALU.mult)

            def finalize_sum():
                # partition-axis sums via one ones-matmul: [G_LOC,1]
                sum_ps = psum.tile([H, HCOLS], F32, tag="mm")
                nc.tensor.matmul(sum_ps[0:G_LOC, 0:1], mmall_sb[:], ones_sb[:],
                                 start=True, stop=True)
                nc.vector.tensor_copy(res_sb[:], sum_ps[0:G_LOC, 0:1])

            # group 0 + first fused-L0s go first so the pipeline starts ASAP;
            # the rest of the setup DMAs overlap with the first halves
            setup_group(0)
            stage_l0(0)
            setup_group(1)
            stage_l0(2)
            setup_group(2)
            setup_group(3)
            nc.sync.dma_start(
                ws_sb[:, H:6 * H].rearrange("p (l c) -> p l c", l=5),
                ws_d[1:6].rearrange("l p c -> p l c"))
            nc.gpsimd.dma_start(wout_sb[:], wout_d[:])
            nc.gpsimd.dma_start(bout_sb[:], bout_d[:])
            nc.vector.memset(ones_sb[:], 1.0)
            for g in range(G_LOC):
                make_mask(g)

            for _pos, t, hu in _events():
                if t == 1:
                    if hu >= 4:    # first two pairs emitted above
                        stage_l0(hu)
                elif t <= 6:
                    stage_mm(hu, t - 1)
                    stage_act(hu, t - 1)
                else:
                    stage_score(hu)

            for g in range(G_LOC):
                finalize_group(g)
            finalize_sum()
            nc.sync.dma_start(out_d[:], res_sb[:])

    nc.compile()
    return nc


def _get_nc(bias_zero):
    if bias_zero not in _cached_nc:
        _cached_nc[bias_zero] = _build_program(bias_zero)
    return _cached_nc[bias_zero]


def _fp16(a):
    return np.ascontiguousarray(a.astype(np.float16))


def _prep_in_maps(input, Ws, bs, W_out, b_out):
    input = np.ascontiguousarray(np.asarray(input, dtype=np.float32))
    Ws = np.asarray(Ws, dtype=np.float32)
    bs = np.asarray(bs, dtype=np.float32)
    W_out = np.asarray(W_out, dtype=np.float32)
    b_out = np.asarray(b_out, dtype=np.float32)

    c2 = np.array([p[2] for p in POLY], dtype=np.float32)  # per-layer clip scale
    WsS = Ws * c2[:, None, None]       # pre-scaled weights (fp16 on device)

    ctx = input[:, :, 0].reshape(G, T, D)
    ent = input[:, :, 1].reshape(G, T, D)
    ctxT = _fp16(ctx.transpose(0, 2, 1))                  # [G, D, T]
    entT = _fp16(ent.transpose(0, 2, 1))
    ctx0 = np.ascontiguousarray(ctx[:, :, 0])             # [G, T] fp32
    ws16 = _fp16(WsS)
    w0b = _fp16(WsS[0][D:H])
    bsT = np.ascontiguousarray(bs.T)                      # [H, 6] raw (ACT bias)
    bs0s = np.ascontiguousarray((bs[0] * c2[0]).reshape(H, 1))
    bsrow = _fp16((bs * c2[:, None]).reshape(1, 6 * H))   # scaled (DVE mm bias)
    wout = _fp16(W_out)
    bout = np.broadcast_to(b_out.reshape(1, 1), (T, 1)).copy()

    in_maps = []
    for k in range(N_CORES):
        sl = slice(k * G_LOC, (k + 1) * G_LOC)
        gin = np.empty((D, 2 * G_LOC * T), dtype=ctxT.dtype)
        for g in range(G_LOC):
            gin[:, (2 * g) * T:(2 * g + 1) * T] = ctxT[k * G_LOC + g]
            gin[:, (2 * g + 1) * T:(2 * g + 2) * T] = entT[k * G_LOC + g]
        in_maps.append({
            "gin": np.ascontiguousarray(gin),
            "ctx0": np.ascontiguousarray(ctx0[sl].T),
            "Ws": ws16,
            "w0b": w0b,
            "bsT": bsT,
            "bs0s": bs0s,
            "bsrow": bsrow,
            "wout": wout,
            "bout": bout,
        })
    return in_maps


def run_traced(trace=False, **inputs):
    """Returns (output [G], exec_time_ns or None)."""
    nc = _get_nc(bias_zero=bool(np.all(np.asarray(inputs["bs"]) == 0)
                                and np.all(np.asarray(inputs["b_out"]) == 0)))
    in_maps = _prep_in_maps(**inputs)
    res = run_bass_kernel_spmd(nc, in_maps, list(range(N_CORES)), trace=trace)
    out = np.concatenate([res.results[k]["out"].reshape(G_LOC)
                          for k in range(N_CORES)])
    return out, res.exec_time_ns


def kernel(**inputs) -> np.ndarray:
    # One rare device-level flake was observed to corrupt a single run, so
    # verify by agreement: run twice; on mismatch run a third time and take
    # the elementwise median.
    out1, _ = run_traced(trace=False, **inputs)
    out2, _ = run_traced(trace=False, **inputs)
    if np.allclose(out1, out2, rtol=1e-4, atol=1e-3):
        return out1
    out3, _ = run_traced(trace=False, **inputs)
    return np.median(np.stack([out1, out2, out3]), axis=0).astype(out1.dtype)
